# revision 33
# baseline (speedup 1.0000x reference)
"""DiffJPEG forward on 8 Trainium2 NeuronCores (Bass/Tile).

Data-parallel over the batch dim (8 batch elems -> 8 cores). Each core
processes one [3,1024,1024] image in 48 tiles of [128 rows, 512 cols]:

  - 2D DCT as two matmul stages:
      MM_A (x4): fused transpose + column(x)-transform, fp32, lhsT = image
      MM_B     : y->v transform, N=512 (fp32r), plus a K=1 rank-1 fp16 matmul
                 adding the -1024 DC bias (exact replacement for the -128
                 pixel offset; the 255x scale folds into the DCT matrix)
  - quantization + differentiable round:
      U  = D*IQ                     (fp32, DVE)
      d' = (U+0.5) pymod 1.0        (= d+0.5, replaces round; DVE)
      d16= d'-0.5 -> fp16           (Pool)
      e16= Square(d'-0.5) -> fp16   (ACT)
      t16= (e16-1)*d16              (fp16 cube correction, DVE)
      q  = U + t16                  ( == round(U) + frac^3 ; Pool)
  - q tiles DMA'd to a DRAM scratch in the natural HW layout; the host
    unshuffles to [B,3,16384,8,8] (pure numpy transpose)
  - dequant W = q*YT (fp16), IDCT as two fp16 matmul stages; 1/255 is folded
    into the second IDCT matrix and the +128/255 offset enters as a K=2
    rank-2 fp16 bias matmul, so rec = min(relu(pO), 1) needs only two
    cheap elementwise ops.

All constants ship in one packed DMA. Built at the bacc layer so its
legalization passes split multi-semaphore waits to satisfy the TRN2
one-wait-per-instruction constraint.
"""

import os
import sys

import numpy as np

for _p in ("/opt/trn_rl_repo", "/root/.axon_site/_ro/trn_rl_repo"):
    if os.path.isdir(_p) and _p not in sys.path:
        sys.path.insert(0, _p)

import concourse.bacc as bacc
import concourse.bass as bass
import concourse.tile as tile
from concourse import mybir
from concourse.bass_utils import run_bass_kernel_spmd

F32 = mybir.dt.float32
F32R = mybir.dt.float32r
F16 = mybir.dt.float16
OP = mybir.AluOpType
AF = mybir.ActivationFunctionType

# Standard JPEG luminance quantization table, transposed as in the DiffJPEG repo.
_Y_TABLE_RAW = np.array([
    [16, 11, 10, 16, 24, 40, 51, 61],
    [12, 12, 14, 19, 26, 58, 60, 55],
    [14, 13, 16, 24, 40, 57, 69, 56],
    [14, 17, 22, 29, 51, 87, 80, 62],
    [18, 22, 37, 56, 68, 109, 103, 77],
    [24, 35, 55, 64, 81, 104, 113, 92],
    [49, 64, 78, 87, 103, 121, 120, 101],
    [72, 92, 95, 98, 112, 100, 103, 99]], dtype=np.float64).T
_FACTOR = 0.4  # quality 80
MAGIC = float(3 * 2**22)  # round-to-nearest-even magic constant (|x| < 2^22)

# packed-constant column offsets (one f32 dram tensor; f16 consts bitcast at the tail)
_F32_RA, _F32_RB, _F32_IQ, _F32_YT = 0, 128, 256, 768
_F16_RC, _F16_RD, _F16_BP, _F16_BF = 0, 128, 256, 384
_F16_BP2, _F16_BF2 = 896, 1024
_F16_COLS = 2048        # f16 elements in the tail region
_PACK_W = 1280 + _F16_COLS // 2   # f32 columns total


def make_consts():
    n = np.arange(8)
    C1 = np.cos((2 * n[:, None] + 1) * n[None, :] * np.pi / 16)  # [spatial, freq]
    a = np.array([1.0 / np.sqrt(2)] + [1.0] * 7)
    CA = C1 * a[None, :] * 0.5 * 255.0   # stage-1 DCT (folds the 255 img scale)
    CB = C1 * a[None, :] * 0.5           # stage-2 DCT / first IDCT half
    CD = C1 * a[None, :] * 0.5 / 255.0   # second IDCT half (folds 1/255)
    I16 = np.eye(16)
    YT = _Y_TABLE_RAW * _FACTOR          # [u, v] as used by the reference
    u_of_f = np.arange(512) % 8
    v_of_p = np.arange(128) % 8

    cpack = np.zeros((128, _PACK_W), np.float32)
    cpack[:, _F32_RA:_F32_RA + 128] = np.kron(I16, CA)           # [(hi,x),(hi,u)]
    cpack[:, _F32_RB:_F32_RB + 128] = np.kron(I16, CB)           # [(wi,y),(wi,v)]
    cpack[:, _F32_IQ:_F32_IQ + 512] = 1.0 / YT[u_of_f[None, :], v_of_p[:, None]]
    cpack[:, _F32_YT:_F32_YT + 512] = YT[u_of_f[None, :], v_of_p[:, None]]

    cf16 = np.zeros((128, _F16_COLS), np.float16)
    cf16[:, _F16_RC:_F16_RC + 128] = np.kron(I16, CB.T)          # [(wi,v),(wi,y)]
    cf16[:, _F16_RD:_F16_RD + 128] = np.kron(I16, CD.T)          # [(hi,u),(hi,x)]
    # rank-1 DC bias: (255x-128) vs 255x differs by -1024 in the DC coefficient
    cf16[0, _F16_BP:_F16_BP + 128] = (v_of_p == 0)
    cf16[0, _F16_BF:_F16_BF + 512] = -1024.0 * (u_of_f == 0)
    # rank-2 +128/255 rec offset, split so fp16 rounding cancels
    h = 128.0 / 255.0
    h1 = np.float16(h)
    h2 = np.float16(h - float(h1))
    cf16[0:2, _F16_BP2:_F16_BP2 + 128] = 1.0
    cf16[0, _F16_BF2:_F16_BF2 + 512] = h1
    cf16[1, _F16_BF2:_F16_BF2 + 512] = h2
    cpack[:, 1280:] = cf16.view(np.float32)   # bitwise f16 payload in f32 storage
    return {"cpack": cpack}


BUFS = {"x": 4, "a": 3, "u": 4, "q": 4, "w": 3, "r": 4,
        "pA": 2, "pD": 3, "pE": 1, "pO": 2}


def kernel_body(nc, tc, outs, ins, n_ch, n_rc, n_cc):
    """Emit the per-core program. outs=(q_d, rec_d), ins=(x_d, cpack_d)."""
    q_d, rec_d = outs
    x_d, cpack_d = ins

    from contextlib import ExitStack
    B = dict(BUFS)
    ctx = ExitStack()
    with ctx:
        constp = ctx.enter_context(tc.tile_pool(name="const", bufs=1))
        xpool = ctx.enter_context(tc.tile_pool(name="xin", bufs=B["x"]))
        apool = ctx.enter_context(tc.tile_pool(name="amid", bufs=B["a"]))
        upool = ctx.enter_context(tc.tile_pool(name="uq", bufs=B["u"]))
        qpool = ctx.enter_context(tc.tile_pool(name="qout", bufs=B["q"]))
        wpool = ctx.enter_context(tc.tile_pool(name="wmid", bufs=B["w"]))
        rpool = ctx.enter_context(tc.tile_pool(name="recw", bufs=B["r"]))
        psA = ctx.enter_context(tc.tile_pool(name="psA", bufs=B["pA"], space="PSUM"))
        psD = ctx.enter_context(tc.tile_pool(name="psD", bufs=B["pD"], space="PSUM"))
        psE = ctx.enter_context(tc.tile_pool(name="psE", bufs=B["pE"], space="PSUM"))
        psO = ctx.enter_context(tc.tile_pool(name="psO", bufs=B["pO"], space="PSUM"))

        cpack = constp.tile([128, _PACK_W], F32)
        nc.sync.dma_start(out=cpack[:], in_=cpack_d)

        ra_s = cpack[:, _F32_RA:_F32_RA + 128]
        rb_s = cpack[:, _F32_RB:_F32_RB + 128]
        iq_s = cpack[:, _F32_IQ:_F32_IQ + 512]
        yt_s = cpack[:, _F32_YT:_F32_YT + 512]
        cf16 = cpack[:, 1280:_PACK_W].bitcast(F16)
        rc_s = cf16[:, _F16_RC:_F16_RC + 128]
        rd_s = cf16[:, _F16_RD:_F16_RD + 128]
        bp_s = cf16[0:1, _F16_BP:_F16_BP + 128]
        bf_s = cf16[0:1, _F16_BF:_F16_BF + 512]
        bp2_s = cf16[0:2, _F16_BP2:_F16_BP2 + 128]
        bf2_s = cf16[0:2, _F16_BF2:_F16_BF2 + 512]

        def phase1(c, ri, ci):
            """load -> DCT -> quant/diff_round -> q (+ q DMA out)."""
            rows = slice(ri * 128, (ri + 1) * 128)
            cols = slice(ci * 512, (ci + 1) * 512)
            xt = xpool.tile([128, 512], F32, tag="xt")
            nc.sync.dma_start(out=xt[:], in_=x_d[c, rows, cols])

            # DCT stage 1 (fused transpose + x-transform), fp32
            pA = psA.tile([128, 512], F32, tag="pA")
            for j in range(4):
                js = slice(j * 128, (j + 1) * 128)
                nc.tensor.matmul(pA[:, js], lhsT=xt[:, js], rhs=ra_s,
                                 start=True, stop=True)
            A = apool.tile([128, 512], F32, tag="A")
            nc.scalar.copy(A[:], pA[:])

            # DCT stage 2 (fp32), N=512, + rank-1 DC bias
            pD = psD.tile([128, 512], F32, tag="pD")
            nc.tensor.matmul(pD[:], lhsT=rb_s,
                             rhs=A[:], start=True, stop=False)
            nc.tensor.matmul(pD[:], lhsT=bp_s, rhs=bf_s, start=False, stop=True)

            # quant + diff_round
            U = upool.tile([128, 512], F32, tag="U")
            nc.vector.tensor_tensor(U[:], pD[:], iq_s, OP.mult)
            r = upool.tile([128, 512], F32, tag="r")
            nc.vector.tensor_scalar(r[:], U[:], MAGIC, MAGIC, OP.add, OP.subtract)
            d16 = upool.tile([128, 512], F16, tag="d16")
            nc.vector.tensor_tensor(d16[:], U[:], r[:], OP.subtract)
            e16 = upool.tile([128, 512], F16, tag="e16")
            nc.scalar.activation(e16[:], d16[:], AF.Square)
            t16 = upool.tile([128, 512], F16, tag="t16")
            nc.vector.scalar_tensor_tensor(t16[:], e16[:], -1.0, d16[:],
                                           OP.add, OP.mult)
            q = qpool.tile([128, 512], F32, tag="q")
            nc.gpsimd.tensor_tensor(q[:], U[:], t16[:], OP.add)
            nc.sync.dma_start(out=q_d[c, ri, ci], in_=q[:])
            return (q, c, ri, ci)

        def phase2(st):
            """dequant -> IDCT -> rec (+ rec DMA out) for an earlier tile."""
            q, c, ri, ci = st
            rows = slice(ri * 128, (ri + 1) * 128)
            cols = slice(ci * 512, (ci + 1) * 512)
            W = wpool.tile([128, 512], F16, tag="W")
            nc.vector.tensor_tensor(W[:, 0:256], q[:, 0:256], yt_s[:, 0:256], OP.mult)
            nc.gpsimd.tensor_tensor(W[:, 256:512], q[:, 256:512], yt_s[:, 256:512], OP.mult)
            pE = psE.tile([128, 512], F32, tag="pE")
            for j in range(4):
                js = slice(j * 128, (j + 1) * 128)
                nc.tensor.matmul(pE[:, js], lhsT=W[:, js], rhs=rc_s,
                                 start=True, stop=True)
            E = wpool.tile([128, 512], F16, tag="E")
            nc.scalar.copy(E[:], pE[:])

            # IDCT stage 2 (fp16, folds 1/255), + rank-2 +128/255 bias
            pO = psO.tile([128, 512], F32, tag="pO")
            nc.tensor.matmul(pO[:], lhsT=rd_s, rhs=E[:], start=True, stop=False)
            nc.tensor.matmul(pO[:], lhsT=bp2_s, rhs=bf2_s, start=False, stop=True)

            # rec = min(relu(pO), 1)
            recA = rpool.tile([128, 512], F32, tag="recA")
            nc.scalar.activation(recA[:], pO[:], AF.Relu)
            rec = rpool.tile([128, 512], F32, tag="rec")
            nc.gpsimd.tensor_scalar(rec[:], recA[:], 1.0, None, OP.min)
            nc.sync.dma_start(out=rec_d[c, rows, cols], in_=rec[:])

        pending = None
        for c in range(n_ch):
            for ri in range(n_rc):
                for ci in range(n_cc):
                    st = phase1(c, ri, ci)
                    if pending is not None:
                        phase2(pending)
                    pending = st
        phase2(pending)


def build_nc(n_ch=3, n_rc=8, n_cc=2):
    nc = bacc.Bacc("TRN2", target_bir_lowering=False, debug=False)
    H, W = n_rc * 128, n_cc * 512
    x_d = nc.dram_tensor("x", [n_ch, H, W], F32, kind="ExternalInput").ap()
    cpack_d = nc.dram_tensor("cpack", [128, _PACK_W], F32, kind="ExternalInput").ap()
    q_d = nc.dram_tensor("q_raw", [n_ch, n_rc, n_cc, 128, 512], F32,
                         kind="ExternalOutput").ap()
    rec_d = nc.dram_tensor("rec", [n_ch, H, W], F32, kind="ExternalOutput").ap()
    with tile.TileContext(nc) as tc:
        kernel_body(nc, tc, (q_d, rec_d), (x_d, cpack_d), n_ch, n_rc, n_cc)
    nc.compile()
    return nc


def unshuffle_q(q_raw, n_rc=8, n_cc=2):
    """[n_ch,n_rc,n_cc,128,512] -> [n_ch, nblocks, 8, 8] (u,v last)."""
    n_ch = q_raw.shape[0]
    S = q_raw.reshape(n_ch, n_rc, n_cc, 16, 8, 4, 16, 8)  # [c,rc,cc,wi,v,j,hi,u]
    q = S.transpose(0, 1, 6, 2, 5, 3, 7, 4)               # [c,rc,hi,cc,j,wi,u,v]
    return np.ascontiguousarray(q.reshape(n_ch, n_rc * 16 * n_cc * 64, 8, 8))


_CACHE = {}


def _get_nc():
    if "nc" not in _CACHE:
        _CACHE["nc"] = build_nc()
    return _CACHE["nc"]


def kernel(x):
    x = np.ascontiguousarray(np.asarray(x), dtype=np.float32)  # [8,3,1024,1024]
    B = x.shape[0]
    nc = _get_nc()
    consts = make_consts()
    in_maps = [dict(consts, x=x[b]) for b in range(B)]
    res = run_bass_kernel_spmd(nc, in_maps, core_ids=list(range(B)))
    y = np.empty((B, 16384, 8, 8), np.float32)
    cb = np.empty((B, 16384, 8, 8), np.float32)
    cr = np.empty((B, 16384, 8, 8), np.float32)
    rec = np.empty((B, 3, 1024, 1024), np.float32)
    for b in range(B):
        qb = unshuffle_q(res.results[b]["q_raw"])
        y[b], cb[b], cr[b] = qb[0], qb[1], qb[2]
        rec[b] = res.results[b]["rec"]
    return (y, cb, cr, rec)
